# revision 21
# baseline (speedup 1.0000x reference)
"""Trainium2 Bass kernel for NeuralTensorLayer (order-1/2/3 polynomial layer).

    out[b,l] = bias[l] + sum_i X[b,i] W1[i,l]
             + sum_ij X[b,i] X[b,j] W2[i,j,l]
             + sum_ijk X[b,i] X[b,j] X[b,k] W3[i,j,k,l]

with B=32768, D=K=32, data-parallel over 8 NeuronCores (4096 rows each).

Strategy (per core):
  * Exploit (i,j) symmetry: only the 528 pairs i<=j are needed against
    host-symmetrized weights W3s[ij,k,l] = W3[i,j,k,l]+W3[j,i,k,l] (i<j),
    cutting the dominant matmul contraction from 1024 -> 528 (+32 X rows).
  * Pair operands arrive host-pregathered per supertile (8 batch tiles) in
    one DRAM block XX[s] = [128, 5x1024 XE | 5x1024 XR] (bf16); the DVE
    builds Z^T[p,b] = X_i X_j (bf16) chunk by chunk, interleaved with the
    per-tile post-processing so no queue ever head-of-line blocks.
  * One matmul group per 128-row tile accumulating into two PSUM tiles:
    big [128,1024] (T3, l-major k) and low [128,64] (out_low = W2s/W1).
    big pool is triple-buffered so the PE never waits on evacuation.
  * Dummy matmuls at kernel start keep the PE busy while the first input
    DMAs land, tripping the HAM clock-gate to 2.4 GHz early.  Startup DMAs
    are issued in need-time order round-robin across the sync/scalar/
    gpsimd queues (each queue's ring serializes issue+transfer).
  * Post per tile: ScalarE copies PSUM->SBUF bf16 into an interleaved
    (l, 34) layout [k<32 big | out_low | junk], DVE multiplies by the
    Xext broadcast (2x mode; the junk column is killed by the 0.0 pad)
    and reduce-sums k (1x) into a per-supertile [128,256] tile written
    back with a single dense DMA.  bias added on host.
"""

import numpy as np
import ml_dtypes
from contextlib import ExitStack

import concourse.bass as bass
import concourse.bacc as bacc
import concourse.tile as tile
from concourse import mybir
from concourse import bass_utils

BF16 = ml_dtypes.bfloat16

B, D, KOUT = 32768, 32, 32
NCORES = 8
BLOC = B // NCORES          # 4096 rows per core
P = 128                     # rows per tile
SUPER = 8                   # tiles per supertile
SP = SUPER * P              # 1024
NSUPER = BLOC // SP         # 4
NPAIRS = D * (D + 1) // 2   # 528
CHUNK_P = [128, 128, 128, 128, 48]  # matmul partitions per chunk
KG = 34                     # k-grid width: 32 order-3 + out_low + junk
NBIG = 1024                 # big psum columns (l*32+k, k<32)
NCOL = KOUT * KG            # 1088 staged columns
XW = D + 2                  # host-padded X width: 32 + [1.0, 0.0]
NDUMMY = 6                  # PE warm-up matmuls (HAM un-throttle)
MINI = 2 * P                # early columns per chunk for tiles 0-1

PAIRS = [(i, j) for i in range(D) for j in range(i, D)]
I_P = np.array([p[0] for p in PAIRS], np.int32)
J_P = np.array([p[1] for p in PAIRS], np.int32)

F32 = mybir.dt.float32
BF = mybir.dt.bfloat16


def _pack_weights(W1, W2, W3):
    W1 = np.asarray(W1, np.float64)
    W2 = np.asarray(W2, np.float64)
    W3 = np.asarray(W3, np.float64)
    Wcat = np.zeros((5, 128, KOUT, KG), np.float64)
    for p, (i, j) in enumerate(PAIRS):
        c, pp = divmod(p, 128)
        if i < j:
            w3 = W3[i, j] + W3[j, i]   # [k, l]
            w2 = W2[i, j] + W2[j, i]   # [l]
        else:
            w3 = W3[i, i]
            w2 = W2[i, i]
        Wcat[c, pp, :, :D] = w3.T
        Wcat[c, pp, :, D] = w2
    for dd in range(D):                # order-1: X rows in chunk 4
        Wcat[4, 16 + dd, :, D] = W1[dd]
    # big part (l-major k, col l*32+k) then low part (out_low col l, pad 0)
    big = Wcat[:, :, :, :D].reshape(5, 128, KOUT * D)
    low = np.zeros((5, 128, 64), np.float64)
    low[:, :, :KOUT] = Wcat[:, :, :, D]
    packed = np.concatenate([big, low], axis=2)  # [5, 128, 1088]
    return packed.astype(np.float32).astype(BF16)


def _build_module():
    nc = bacc.Bacc("TRN2", target_bir_lowering=False, debug=False,
                   enable_asserts=False)
    XXd = nc.dram_tensor("XX", [NSUPER, 128, 10 * SP], BF, kind="ExternalInput").ap()
    XBDd = nc.dram_tensor("XBD", [NSUPER, 128, SUPER * XW], BF, kind="ExternalInput").ap()
    XTd = nc.dram_tensor("XT", [D, BLOC], BF, kind="ExternalInput").ap()
    WCd = nc.dram_tensor("WCAT", [5, 128, NCOL], BF, kind="ExternalInput").ap()
    OUTd = nc.dram_tensor("OUT", [NSUPER, 128, SUPER * KOUT], F32, kind="ExternalOutput").ap()

    XE_OFF = [c * SP for c in range(5)]
    XR_OFF = [5 * SP + c * SP for c in range(5)]

    with ExitStack() as ctx:
        tc = ctx.enter_context(tile.TileContext(nc))
        consts = ctx.enter_context(tc.tile_pool(name="consts", bufs=1))
        xxpool = ctx.enter_context(tc.tile_pool(name="xxpool", bufs=3))
        xbpool = ctx.enter_context(tc.tile_pool(name="xbpool", bufs=3))
        zpool = ctx.enter_context(tc.tile_pool(name="zpool", bufs=2))
        spool = ctx.enter_context(tc.tile_pool(name="spool", bufs=8))
        upool = ctx.enter_context(tc.tile_pool(name="upool", bufs=5))
        opool = ctx.enter_context(tc.tile_pool(name="opool", bufs=2))
        bigps = ctx.enter_context(tc.tile_pool(name="bigps", bufs=3, space="PSUM"))
        lowps = ctx.enter_context(tc.tile_pool(name="lowps", bufs=2, space="PSUM"))

        g = consts.tile([128, 640], BF, tag="g")
        nc.gpsimd.memset(g, 0.0)

        w_sb = []
        for c in range(5):
            w = consts.tile([128, NCOL], BF, tag=f"w_{c}")
            w_sb.append(w)
        xx_tiles = {0: xxpool.tile([128, 10 * SP], BF, tag="xx", name="xx0")}
        xbd = [None] * NSUPER
        xbd[0] = xbpool.tile([128, SUPER * XW], BF, tag="xbd", name="xbd0")

        # ---- startup DMAs: sync carries the weights-c0 + XE stream, the
        # gpsimd (SWDGE) queue carries w2/w4 + the XR stream, scalar only
        # w1/w3 (+XT rows) so its queue stays clear for the evacuations.
        xx0 = xx_tiles[0]
        nc.sync.dma_start(out=w_sb[0], in_=WCd[0])
        nc.scalar.dma_start(out=w_sb[1], in_=WCd[1])
        nc.gpsimd.dma_start(out=w_sb[2], in_=WCd[2])
        nc.gpsimd.dma_start(out=w_sb[4], in_=WCd[4])
        for c in range(5):
            e0, r0 = XE_OFF[c], XR_OFF[c]
            nc.sync.dma_start(out=xx0[:, e0:e0 + MINI], in_=XXd[0][:, e0:e0 + MINI])
            nc.gpsimd.dma_start(out=xx0[:, r0:r0 + MINI], in_=XXd[0][:, r0:r0 + MINI])
        nc.scalar.dma_start(out=w_sb[3], in_=WCd[3])
        for c in range(5):
            e0, r0 = XE_OFF[c] + MINI, XR_OFF[c] + MINI
            nc.sync.dma_start(out=xx0[:, e0:e0 + SP - MINI],
                              in_=XXd[0][:, e0:e0 + SP - MINI])
            nc.gpsimd.dma_start(out=xx0[:, r0:r0 + SP - MINI],
                                in_=XXd[0][:, r0:r0 + SP - MINI])
        nc.sync.dma_start(out=xbd[0], in_=XBDd[0])

        # PE warm-up: results are discarded (start=True clears the bank for
        # the first real accumulation into the same buffers later).
        for _ in range(NDUMMY):
            dummy = bigps.tile([128, NBIG], F32, tag="big")
            nc.tensor.matmul(dummy[:, 0:512], g[:, :128], g[:, 128:640],
                             start=True, stop=True)

        def z_tiles(sfx):
            zs = []
            for c in range(5):
                z = zpool.tile([CHUNK_P[c], SP], BF, tag=f"z{c}", name=f"z{c}{sfx}")
                zs.append(z)
            return zs

        def z_mul(s, zs, c, a, b):
            """pair products for column range [a,b) of chunk c (vector)."""
            xx = xx_tiles[s]
            pc = 128 if c < 4 else 16
            nc.vector.tensor_mul(zs[c][:pc, a:b],
                                 xx[:pc, XE_OFF[c] + a:XE_OFF[c] + b],
                                 xx[:pc, XR_OFF[c] + a:XR_OFF[c] + b])

        def z_xrows(s, zs, a, b):
            """order-1 X rows -> partitions 16:48 of chunk 4 (after z_mul)."""
            nc.scalar.dma_start(out=zs[4][16:48, a:b],
                                in_=XTd[:, s * SP + a:s * SP + b])

        def fetch_xe(s):
            """sync-queue burst: XE chunks + XBD for supertile s."""
            xx = xxpool.tile([128, 10 * SP], BF, tag="xx", name=f"xx{s}")
            xx_tiles[s] = xx
            for c in range(5):
                e0 = XE_OFF[c]
                nc.sync.dma_start(out=xx[:, e0:e0 + SP], in_=XXd[s][:, e0:e0 + SP])
            xbd[s] = xbpool.tile([128, SUPER * XW], BF, tag="xbd", name=f"xbd{s}")
            nc.sync.dma_start(out=xbd[s], in_=XBDd[s])

        def fetch_xr_chunk(s, c):
            """one XR chunk via the gpsimd (SWDGE) queue, keeping the scalar
            queue free for the PSUM evacuations."""
            xx = xx_tiles[s]
            r0 = XR_OFF[c]
            nc.gpsimd.dma_start(out=xx[:, r0:r0 + SP], in_=XXd[s][:, r0:r0 + SP])

        # z for supertile 0: tiles 0-1 columns now, the rest interleaved
        # into the tile-0/1 post-ops below as its DMAs land.
        zs0 = z_tiles("s0")
        for c in range(5):
            z_mul(0, zs0, c, 0, MINI)
        z_xrows(0, zs0, 0, MINI)
        # supertile 1 inputs: XE burst on sync, XR chunks via gpsimd
        if NSUPER > 1:
            fetch_xe(1)
            for c in range(5):
                fetch_xr_chunk(1, c)
        zs_cur = zs0

        for s in range(NSUPER):
            zs_next = z_tiles(f"s{s + 1}") if s + 1 < NSUPER else None
            osb = opool.tile([128, SUPER * KOUT], F32, tag="osb")
            for t in range(SUPER):
                big = bigps.tile([128, NBIG], F32, tag="big")
                low = lowps.tile([128, 64], F32, tag="low")
                for c in range(5):
                    pcp = CHUNK_P[c]
                    st = zs_cur[c][:pcp, t * P:(t + 1) * P]
                    first, last = c == 0, c == 4
                    nc.tensor.matmul(big[:, 0:512], st, w_sb[c][:pcp, 0:512],
                                     start=first, stop=last)
                    nc.tensor.matmul(big[:, 512:1024], st, w_sb[c][:pcp, 512:1024],
                                     start=first, stop=last)
                    nc.tensor.matmul(low, st, w_sb[c][:pcp, 1024:1088],
                                     start=first, stop=last)
                # staged layout is (l, 34): k<32 from big, k=32 out_low,
                # k=33 junk (killed by the 0.0 pad in xbd).
                staged2 = spool.tile([128, NCOL], BF, tag="staged2")
                stv = staged2[:, :].rearrange("p (l k) -> p l k", k=KG)
                nc.scalar.copy(out=stv[:, :, D:KG],
                               in_=low[:, :].rearrange("p (k l) -> p l k", k=2))
                nc.scalar.copy(out=stv[:, :, 0:D],
                               in_=big[:, :].rearrange("p (l k) -> p l k", k=D))
                u = upool.tile([128, NCOL], BF, tag="u")
                xk = (xbd[s][:, t * XW:(t + 1) * XW]
                      .unsqueeze(1).broadcast_to([P, KOUT, XW]))
                nc.vector.tensor_mul(
                    u[:, :].rearrange("p (l k) -> p l k", k=KG), stv, xk)
                nc.vector.reduce_sum(
                    out=osb[:, t * KOUT:(t + 1) * KOUT],
                    in_=u[:, :].rearrange("p (l k) -> p l k", k=KG),
                    axis=mybir.AxisListType.X)
                # supertile 0 only: build the remaining z columns as the
                # rest DMAs land, without blocking the post-op stream.
                if s == 0 and t <= 1:
                    for c in (0, 1) if t == 0 else (2, 3, 4):
                        z_mul(0, zs0, c, MINI, SP)
                    if t == 1:
                        z_xrows(0, zs0, MINI, SP)
                # spread next-supertile work through this supertile's tiles:
                # XE burst + XR chunks on the DMA queues, z products on the
                # vector queue once their inputs have landed.
                if s + 2 < NSUPER:
                    if t == 0:
                        fetch_xe(s + 2)
                    if t <= 4:
                        fetch_xr_chunk(s + 2, t)
                if zs_next is not None and 2 <= t <= 6:
                    z_mul(s + 1, zs_next, t - 2, 0, SP)
                    if t == 6:
                        z_xrows(s + 1, zs_next, 0, SP)
            nc.sync.dma_start(out=OUTd[s], in_=osb)
            zs_cur = zs_next
    nc.compile()
    return nc


_CACHE = {}


def _get_module():
    if "nc" not in _CACHE:
        _CACHE["nc"] = _build_module()
    return _CACHE["nc"]


def kernel(X, W1, W2, W3, bias):
    X = np.ascontiguousarray(np.asarray(X, np.float32))
    bias = np.asarray(bias, np.float32)
    Wcat = _pack_weights(W1, W2, W3)

    nc = _get_module()
    Xb = X.astype(BF16)                      # [B, D] bf16 (single rounding point)
    XbT = np.ascontiguousarray(Xb.T)         # [D, B] bf16
    npad = 5 * 128 - NPAIRS
    XE = np.concatenate([XbT[I_P], np.zeros((npad, B), BF16)], 0).reshape(5, 128, B)
    XR = np.concatenate([XbT[J_P], np.zeros((npad, B), BF16)], 0).reshape(5, 128, B)
    Xpad = np.zeros((B, XW), BF16)
    Xpad[:, :D] = Xb
    Xpad[:, D] = BF16(1.0)

    in_maps = []
    for c in range(NCORES):
        lo, hi = c * BLOC, (c + 1) * BLOC
        xe = (XE[:, :, lo:hi].reshape(5, 128, NSUPER, SP)
              .transpose(2, 1, 0, 3).reshape(NSUPER, 128, 5 * SP))
        xr = (XR[:, :, lo:hi].reshape(5, 128, NSUPER, SP)
              .transpose(2, 1, 0, 3).reshape(NSUPER, 128, 5 * SP))
        xx = np.concatenate([xe, xr], axis=2)        # [NSUPER, 128, 10*SP]
        xbd = (Xpad[lo:hi].reshape(NSUPER, SUPER, P, XW)
               .transpose(0, 2, 1, 3).reshape(NSUPER, 128, SUPER * XW))
        in_maps.append({
            "XX": np.ascontiguousarray(xx),
            "XBD": np.ascontiguousarray(xbd),
            "XT": np.ascontiguousarray(XbT[:, lo:hi]),
            "WCAT": Wcat,
        })
    res = bass_utils.run_bass_kernel_spmd(nc, in_maps, core_ids=list(range(NCORES)))
    _CACHE["last_results"] = res
    outs = []
    for c in range(NCORES):
        od = np.asarray(res.results[c]["OUT"])       # [NSUPER, 128, SUPER*KOUT]
        outs.append(od.reshape(NSUPER, P, SUPER, KOUT)
                    .transpose(0, 2, 1, 3).reshape(BLOC, KOUT))
    out = np.concatenate(outs, 0)
    return (out + bias.reshape(1, KOUT)).astype(np.float32)
